# revision 29
# baseline (speedup 1.0000x reference)
"""Routed per-behavior FFN (MoE-style) Trainium2 kernel.

Reference semantics: for each token t with b = b_seq[t]:
  b == 0      -> output 0
  b in 1..4   -> LN(elu(x W1_b^T + b1_b) W2_b^T + b2_b) * gamma_b + beta_b

Strategy (v2):
- Host routing (metadata only): tokens are sorted by branch and each
  branch's token list is split evenly over the 8 cores, so every core runs
  an identical-shape grouped FFN over ~1/8 of the routed tokens. Gather of
  x and the final scatter are host-side shard/unshard steps.
- All matmul operands in bf16 (fp32 PSUM accumulation): halves HBM+SBUF
  traffic and enables fast-weight-load so LDWEIGHTS hides behind matmuls
  (the fp32r baseline exposed ~40us of weight loads).
- ELU = relu(v) + (min(exp(v),1) - 1), split across three engines:
  ScalarE exp (PSUM->SBUF), GPSIMD tensor_scalar for min/sub (otherwise
  idle), DVE scalar_tensor_tensor for the final max/add from PSUM.
- LayerNorm: mean arrives as a 257th output column of the W2 matmul
  (host appends colsum(W2)/256); variance = Sum(y^2)/256 - mu^2 with the
  squared sum from one ScalarE Square+accum_out pass; rstd computed as
  exp(-0.5*ln(var+eps)) so every ScalarE function (exp/ln/square/identity)
  lives in ONE activation table set - zero table swaps (the sqrt-based
  baseline paid ~13us in ACT_TABLE_LOADs). The normalize is a single
  fused (y - mu)*rstd op per token tile with per-partition scalars.
- Chunk-level software pipeline: W2+LN of chunk c-1 is emitted interleaved
  with W1+ELU of chunk c so the Tensor engine always has independent work
  while the elementwise chain drains; x/weights DMAs prefetch one chunk
  ahead.
"""

import json
import os

import numpy as np

B, T = 32, 2048
D_MODEL = 256
D_FF = 1024
N_B = 4
NCORES = 8
LN_EPS = 1e-12
NTOK = B * T

# ---------------------------------------------------------------------------
# walrus workaround: this container's compiler accepts at most one sync wait
# per CTRL-class instruction; split extras onto NoOp carriers.
# ---------------------------------------------------------------------------


def _split_excess_waits(bir: dict, max_waits: int = 1) -> None:
    for fn in bir.get("functions", []):
        for blk in fn.get("blocks", []):
            insts = blk.get("instructions")
            if not insts:
                continue
            new = []
            for inst in insts:
                si = inst.get("sync_info")
                waits = (si or {}).get("on_wait") or []
                if len(waits) > max_waits:
                    excess, keep = waits[:-max_waits], waits[-max_waits:]
                    for k, w in enumerate(excess):
                        new.append(
                            {
                                "debug": inst.get("debug", 0),
                                "engine": inst["engine"],
                                "ins": [],
                                "name": f"{inst['name']}-wsplit{k}",
                                "opcode": "NoOp",
                                "outs": [],
                                "sync_info": {"on_update": [], "on_wait": [w]},
                            }
                        )
                    si["on_wait"] = keep
                new.append(inst)
            blk["instructions"] = new


_bir_fix_installed = False


def _install_bir_fix():
    global _bir_fix_installed
    if _bir_fix_installed:
        return
    import concourse.bass_utils as bass_utils
    import concourse.bass2jax as bass2jax

    orig = bass_utils.compile_bir_kernel

    if os.environ.get("LDW_OPT"):
        _orig_bvo = bass_utils.bir_verify_and_optimise

        def _bvo(tmpdir, inp="bir.json", outp="file.neff", arch=None, **kw):
            import unittest.mock as _mock

            real_run = bass_utils.run_command

            def run2(argv, **kwargs):
                argv = [
                    a.replace("--enable-ldw-opt=false", "--enable-ldw-opt=true")
                    for a in argv
                ]
                return real_run(argv, **kwargs)

            with _mock.patch.object(bass_utils, "run_command", run2):
                return _orig_bvo(tmpdir, inp, outp, arch, **kw)

        bass_utils.bir_verify_and_optimise = _bvo

    def patched(bir_json, tmpdir, neff_name="file.neff"):
        bir = json.loads(bir_json)
        _split_excess_waits(bir)
        return orig(json.dumps(bir).encode(), tmpdir, neff_name)

    bass_utils.compile_bir_kernel = patched
    bass2jax.compile_bir_kernel = patched

    # Synthesize antenv.axon_hooks (absent in this image) so trace=True can
    # reach the terminal's NTFF profiler via the axon .so.
    import sys
    import types

    if "antenv.axon_hooks" not in sys.modules:
        try:
            from trn_agent_boot.trn_boot import _ntff_profile_via_ctypes

            hook = _ntff_profile_via_ctypes("/opt/axon/libaxon_pjrt.so")
            mod = types.ModuleType("antenv.axon_hooks")
            mod.get_axon_ntff_profile_hook = lambda: hook
            mod.set_axon_ntff_profile_hook = lambda h: None
            sys.modules["antenv.axon_hooks"] = mod
        except Exception:
            pass
    _bir_fix_installed = True


# ---------------------------------------------------------------------------
# device kernel builder
# ---------------------------------------------------------------------------

_BUILD_CACHE = {}


def _chunks(cap, w=512):
    out = []
    off = 0
    while off < cap:
        out.append((off, min(w, cap - off)))
        off += w
    return out


def _build(caps, b1_nonzero, b2_nonzero, gb_nontrivial):
    kedt = os.environ.get("KEDT", "bf16")  # dtype of exp/neg intermediates
    kfs = int(os.environ.get("KFS", "0"))  # finals per chunk on ScalarE
    ka2 = int(os.environ.get("KA2", "0"))  # ELU groups per chunk on relu path
    key = (tuple(caps), b1_nonzero, b2_nonzero, gb_nontrivial, kedt, kfs, ka2)
    if key in _BUILD_CACHE:
        return _BUILD_CACHE[key]

    import concourse.bass as bass
    import concourse.tile as tile
    from concourse import mybir

    f32 = mybir.dt.float32
    bf16 = mybir.dt.bfloat16
    edt = bf16 if kedt == "bf16" else f32
    S = sum(caps)
    KD = D_MODEL // 128  # 2 chunks of the model dim
    KF = D_FF // 128  # 8 chunks of the ff dim
    D2 = D_MODEL + 2  # mean column + pad

    nc = bass.Bass("TRN2")
    xg = nc.dram_tensor("xg", [KD, 128, S], bf16, kind="ExternalInput")
    w1t = nc.dram_tensor("w1t", [N_B, KD, 128, D_FF], bf16, kind="ExternalInput")
    w2t = nc.dram_tensor("w2t", [N_B, KF, 128, D2], bf16, kind="ExternalInput")
    if b2_nonzero:
        b2e = nc.dram_tensor("b2e", [N_B, D2], bf16, kind="ExternalInput")
    if b1_nonzero:
        b1d = nc.dram_tensor("b1", [N_B, D_FF], bf16, kind="ExternalInput")
    if gb_nontrivial:
        gamd = nc.dram_tensor("gamma", [N_B, D_MODEL], f32, kind="ExternalInput")
        betd = nc.dram_tensor("beta", [N_B, D_MODEL], f32, kind="ExternalInput")
    yc = nc.dram_tensor("yc", [S, D_MODEL], bf16, kind="ExternalOutput")

    AF = mybir.ActivationFunctionType
    OP = mybir.AluOpType

    # flat chunk list across branches
    descs = []
    seg_off = 0
    for n in range(N_B):
        cap = caps[n]
        if cap == 0:
            continue
        for off, W in _chunks(cap):
            descs.append((n, off, W, seg_off + off, off == 0))
        seg_off += cap

    with tile.TileContext(nc) as tc:
        with (
            tc.tile_pool(name="singles", bufs=1) as singles,
            tc.tile_pool(name="w1p", bufs=2) as w1p,
            tc.tile_pool(name="w2p", bufs=2) as w2p,
            tc.tile_pool(name="cns", bufs=2) as cns,
            tc.tile_pool(name="xp", bufs=4) as xp,
            tc.tile_pool(name="ep", bufs=4) as ep,
            tc.tile_pool(name="ngp", bufs=4) as ngp,
            tc.tile_pool(name="hp", bufs=3) as hp,
            tc.tile_pool(name="op_", bufs=4) as op_,
            tc.tile_pool(name="scrp", bufs=2) as scrp,
            tc.tile_pool(name="stp", bufs=6) as stp,
            tc.tile_pool(name="php", bufs=2, space="PSUM") as php,
            tc.tile_pool(name="pyp", bufs=4, space="PSUM") as pyp,
        ):
            eps_tile = singles.tile([128, 1], f32)
            nc.vector.memset(eps_tile, LN_EPS)
            if b1_nonzero:
                ones_row = singles.tile([1, 512], bf16)
                nc.vector.memset(ones_row, 1.0)
            if b2_nonzero:
                ones_col = singles.tile([1, 128], bf16)
                nc.vector.memset(ones_col, 1.0)

            bstate = {}  # branch -> weight tiles
            xg_tiles = {}  # chunk idx -> xg tile

            def emit_weights(i):
                if i >= len(descs):
                    return
                n, off, W, goff, first = descs[i]
                if first:
                    w1_sb = w1p.tile([128, KD, D_FF], bf16, tag="w1")
                    for k in range(KD):
                        nc.sync.dma_start(out=w1_sb[:, k, :], in_=w1t[n, k])
                    b1_sb = b2e_sb = gam_bc = bet_bc = None
                    if b1_nonzero:
                        b1_sb = cns.tile([1, D_FF], bf16, tag="b1")
                        nc.sync.dma_start(out=b1_sb, in_=b1d[n : n + 1, :])
                    if b2_nonzero:
                        b2e_sb = cns.tile([1, D2], bf16, tag="b2e")
                        nc.sync.dma_start(out=b2e_sb, in_=b2e[n : n + 1, :])
                    if gb_nontrivial:
                        gam_bc = cns.tile([128, D_MODEL], f32, tag="gam")
                        bet_bc = cns.tile([128, D_MODEL], f32, tag="bet")
                        gsrc = gamd[n : n + 1, :]
                        bsrc = betd[n : n + 1, :]
                        nc.gpsimd.dma_start(
                            out=gam_bc,
                            in_=bass.AP(
                                tensor=gsrc.tensor,
                                offset=gsrc.offset,
                                ap=[[0, 128], gsrc.ap[1]],
                            ),
                        )
                        nc.gpsimd.dma_start(
                            out=bet_bc,
                            in_=bass.AP(
                                tensor=bsrc.tensor,
                                offset=bsrc.offset,
                                ap=[[0, 128], bsrc.ap[1]],
                            ),
                        )
                    bstate[n] = [w1_sb, None, b1_sb, b2e_sb, gam_bc, bet_bc]

            def emit_w2(i):
                if i >= len(descs):
                    return
                n, off, W, goff, first = descs[i]
                if first:
                    w2_sb = w2p.tile([128, KF, D2], bf16, tag="w2")
                    nc.sync.dma_start(
                        out=w2_sb, in_=w2t[n].rearrange("j p d -> p j d")
                    )
                    bstate[n][1] = w2_sb

            def emit_x(i):
                if i >= len(descs):
                    return
                n, off, W, goff, first = descs[i]
                xg_sb = xp.tile([128, KD, 512], bf16, tag="xg")
                nc.sync.dma_start(
                    out=xg_sb[:, :, :W],
                    in_=xg[:, :, goff : goff + W].rearrange("k p w -> p k w"),
                )
                xg_tiles[i] = xg_sb

            def phase1(i):
                """W1 matmuls + ELU for chunk i; yields once per f-group."""
                emit_weights(i + 1)
                emit_x(i + 1)
                emit_w2(i + 1)
                n, off, W, goff, first = descs[i]
                w1_sb, w2_sb, b1_sb, b2e_sb, gam_bc, bet_bc = bstate[n]
                xg_sb = xg_tiles.pop(i)
                h_sb = hp.tile([128, KF, 512], bf16, tag="h")
                phs = {}
                for g in range(KF // 2):
                    ph = php.tile([128, 2, 512], f32, tag="ph")
                    phs[g] = ph
                    for fi in range(2):
                        f = g * 2 + fi
                        fs = slice(f * 128, (f + 1) * 128)
                        nc.tensor.matmul(
                            ph[:, fi, :W],
                            w1_sb[:, 0, fs],
                            xg_sb[:, 0, :W],
                            start=True,
                            stop=False,
                        )
                        nc.tensor.matmul(
                            ph[:, fi, :W],
                            w1_sb[:, 1, fs],
                            xg_sb[:, 1, :W],
                            start=False,
                            stop=not b1_nonzero,
                        )
                        if b1_nonzero:
                            nc.tensor.matmul(
                                ph[:, fi, :W],
                                b1_sb[:, fs],
                                ones_row[:, :W],
                                start=False,
                                stop=True,
                            )
                    # elu(v) = max(v,0) + (min(exp(v),1) - 1)
                    if g % 2 == 0:
                        e_sb = ep.tile([128, 4, 512], edt, tag="e")
                        ng_sb = ngp.tile([128, 4, 512], edt, tag="ng")
                    gi = g % 2
                    nc.scalar.activation(
                        e_sb[:, gi * 2 : gi * 2 + 2, :W], ph[:, :, :W], AF.Exp
                    )
                    if g % 2 == 1:
                        # one min/sub over both groups of the pair
                        nc.vector.tensor_scalar(
                            ng_sb[:, :, :W],
                            e_sb[:, :, :W],
                            scalar1=1.0,
                            scalar2=1.0,
                            op0=OP.min,
                            op1=OP.subtract,
                        )
                        for gg in (g - 1, g):
                            ggi = gg % 2
                            nc.vector.scalar_tensor_tensor(
                                h_sb[:, gg * 2 : gg * 2 + 2, :W],
                                phs[gg][:, :, :W],
                                0.0,
                                ng_sb[:, ggi * 2 : ggi * 2 + 2, :W],
                                op0=OP.max,
                                op1=OP.add,
                            )
                    yield
                xg_tiles[("h", i)] = h_sb

            def phase2(i):
                """W2 matmuls + LN + store for chunk i; yields per token tile."""
                n, off, W, goff, first = descs[i]
                w1_sb, w2_sb, b1_sb, b2e_sb, gam_bc, bet_bc = bstate[n]
                h_sb = xg_tiles.pop(("h", i))
                nW = W // 128
                mu_neg = stp.tile([128, 4], f32, tag="mu")
                varall = stp.tile([128, 4], f32, tag="var")
                aux = stp.tile([128, 12], f32, tag="aux")
                m2, tv, lnv = aux[:, 0:4], aux[:, 4:8], aux[:, 8:12]
                rstd = stp.tile([128, 4], f32, tag="rstd")
                murstd = stp.tile([128, 4], f32, tag="mrs")
                o2 = op_.tile([128, 4, D_MODEL], bf16, tag="o2")
                pys = []
                for t in range(nW):
                    py = pyp.tile([128, 512], f32, tag="py")
                    for f in range(KF):
                        nc.tensor.matmul(
                            py[:, :D2],
                            h_sb[:, f, t * 128 : (t + 1) * 128],
                            w2_sb[:, f, :],
                            start=(f == 0),
                            stop=not b2_nonzero and f == KF - 1,
                        )
                    if b2_nonzero:
                        nc.tensor.matmul(
                            py[:, :D2],
                            ones_col[:, :128],
                            b2e_sb,
                            start=False,
                            stop=True,
                        )
                    scr = scrp.tile([128, D_MODEL], bf16, tag="scr")
                    nc.scalar.activation(
                        scr,
                        py[:, :D_MODEL],
                        AF.Square,
                        accum_out=varall[:, t : t + 1],
                    )
                    # m2 = -256 * mu^2 (mu read straight out of PSUM col 256)
                    nc.vector.tensor_scalar(
                        m2[:, t : t + 1],
                        py[:, D_MODEL : D_MODEL + 1],
                        scalar1=py[:, D_MODEL : D_MODEL + 1],
                        scalar2=-float(D_MODEL),
                        op0=OP.mult,
                        op1=OP.mult,
                    )
                    if t < kfs:
                        nc.vector.tensor_scalar_mul(
                            mu_neg[:, t : t + 1],
                            py[:, D_MODEL : D_MODEL + 1],
                            scalar1=-1.0,
                        )
                    pys.append(py)
                    # finish LN for a pair of token tiles as soon as their
                    # stats are in, so their PSUM banks free up early.
                    if t % 2 == 1 or t == nW - 1:
                        lo = t & ~1
                        hi = t + 1
                        sl = slice(lo, hi)
                        # rstd = exp(-0.5 * ln(sum(y^2)/256 - mu^2 + eps))
                        nc.vector.tensor_tensor(
                            tv[:, sl], varall[:, sl], m2[:, sl], OP.add
                        )
                        nc.scalar.activation(
                            lnv[:, sl],
                            tv[:, sl],
                            AF.Ln,
                            bias=eps_tile,
                            scale=1.0 / D_MODEL,
                        )
                        nc.scalar.activation(
                            rstd[:, sl], lnv[:, sl], AF.Exp, scale=-0.5
                        )
                        if kfs > 0:
                            nc.gpsimd.tensor_tensor(
                                murstd[:, sl], mu_neg[:, sl], rstd[:, sl], OP.mult
                            )
                        for u in range(lo, hi):
                            if u < kfs:
                                nc.scalar.activation(
                                    o2[:, u, :],
                                    pys[u][:, :D_MODEL],
                                    AF.Identity,
                                    bias=murstd[:, u : u + 1],
                                    scale=rstd[:, u : u + 1],
                                )
                            else:
                                nc.vector.tensor_scalar(
                                    o2[:, u, :],
                                    pys[u][:, :D_MODEL],
                                    scalar1=pys[u][:, D_MODEL : D_MODEL + 1],
                                    scalar2=rstd[:, u : u + 1],
                                    op0=OP.subtract,
                                    op1=OP.mult,
                                )
                            if gb_nontrivial:
                                nc.vector.tensor_mul(
                                    o2[:, u, :], o2[:, u, :], gam_bc
                                )
                                nc.vector.tensor_add(
                                    o2[:, u, :], o2[:, u, :], bet_bc
                                )
                    yield
                nc.sync.dma_start(
                    out=yc[goff : goff + W, :].rearrange("(c p) d -> p c d", p=128),
                    in_=o2[:, :nW, :],
                )
                yield

            emit_weights(0)
            emit_x(0)
            emit_w2(0)
            prev = None
            for i in range(len(descs)):
                p1 = phase1(i)
                if prev is None:
                    for _ in p1:
                        pass
                else:
                    p2 = phase2(prev)
                    gens = [p2, p1]
                    gi = 0
                    while gens:
                        g = gens[gi % len(gens)]
                        try:
                            next(g)
                            gi += 1
                        except StopIteration:
                            gens.remove(g)
                prev = i
            for _ in phase2(prev):
                pass

    _BUILD_CACHE[key] = nc
    return nc


# ---------------------------------------------------------------------------
# host wrapper
# ---------------------------------------------------------------------------


def kernel(x, b_seq, w1, b1, w2, b2, gamma, beta):
    _install_bir_fix()
    import ml_dtypes
    from concourse.bass_utils import run_bass_kernel_spmd

    bf16 = ml_dtypes.bfloat16
    x = np.asarray(x, dtype=np.float32)
    b_seq = np.asarray(b_seq, dtype=np.int32)
    w1 = np.asarray(w1, dtype=np.float32)
    b1 = np.asarray(b1, dtype=np.float32)
    w2 = np.asarray(w2, dtype=np.float32)
    b2 = np.asarray(b2, dtype=np.float32)
    gamma = np.asarray(gamma, dtype=np.float32)
    beta = np.asarray(beta, dtype=np.float32)

    x_flat = x.reshape(NTOK, D_MODEL)
    bs = b_seq.reshape(NTOK)

    # token ids per branch, split evenly over cores
    parts = []  # parts[n][c] -> int array of token ids
    for n in range(1, N_B + 1):
        idx = np.nonzero(bs == n)[0].astype(np.int64)
        parts.append(np.array_split(idx, NCORES))
    caps = []
    for n in range(N_B):
        mx = max(len(p) for p in parts[n])
        caps.append(0 if mx == 0 else ((mx + 127) // 128) * 128)
    S = sum(caps)
    D2 = D_MODEL + 2

    b1_nonzero = bool(np.any(b1))
    b2_nonzero = bool(np.any(b2))
    gb_nontrivial = bool(np.any(beta)) or not bool(np.all(gamma == 1.0))

    nc = _build(tuple(caps), b1_nonzero, b2_nonzero, gb_nontrivial)

    # weight layouts
    w1t = np.ascontiguousarray(
        w1.transpose(0, 2, 1).reshape(N_B, D_MODEL // 128, 128, D_FF)
    ).astype(bf16)
    w2t_core = w2.transpose(0, 2, 1).reshape(N_B, D_FF // 128, 128, D_MODEL)
    w2_colmean = (w2.sum(axis=1) / D_MODEL).reshape(N_B, D_FF // 128, 128, 1)
    w2t = np.ascontiguousarray(
        np.concatenate(
            [w2t_core, w2_colmean, np.zeros_like(w2_colmean)], axis=3
        )
    ).astype(bf16)
    b2e = np.ascontiguousarray(
        np.concatenate(
            [
                b2,
                b2.sum(axis=1, keepdims=True) / D_MODEL,
                np.zeros((N_B, 1), np.float32),
            ],
            axis=1,
        )
    ).astype(bf16)

    in_maps = []
    for c in range(NCORES):
        gidx = np.zeros(S, dtype=np.int64)
        seg = 0
        for n in range(N_B):
            p = parts[n][c]
            gidx[seg : seg + len(p)] = p
            seg += caps[n]
        xgc = np.ascontiguousarray(
            x_flat[gidx].T.reshape(D_MODEL // 128, 128, S)
        ).astype(bf16)
        m = {"xg": xgc, "w1t": w1t, "w2t": w2t}
        if b2_nonzero:
            m["b2e"] = b2e
        if b1_nonzero:
            m["b1"] = np.ascontiguousarray(b1).astype(bf16)
        if gb_nontrivial:
            m["gamma"] = gamma
            m["beta"] = beta
        in_maps.append(m)

    import time

    trace = bool(os.environ.get("KERNEL_TRACE"))
    res = None
    for attempt in range(3):
        try:
            res = run_bass_kernel_spmd(
                nc, in_maps, core_ids=list(range(NCORES)), trace=trace
            )
            break
        except Exception:
            # transient NRT device errors have been observed on the first
            # execution of a freshly compiled NEFF; retry
            if attempt == 2:
                raise
            time.sleep(3)
    global LAST_RESULTS
    LAST_RESULTS = res

    out_flat = np.zeros((NTOK, D_MODEL), dtype=np.float32)
    for c in range(NCORES):
        ycc = np.asarray(res.results[c]["yc"]).astype(np.float32)
        seg = 0
        for n in range(N_B):
            p = parts[n][c]
            out_flat[p] = ycc[seg : seg + len(p)]
            seg += caps[n]
    return out_flat.reshape(B, T, D_MODEL)


# revision 31
# speedup vs baseline: 1.3605x; 1.3605x over previous
"""Routed per-behavior FFN (MoE-style) Trainium2 kernel.

Reference semantics: for each token t with b = b_seq[t]:
  b == 0      -> output 0
  b in 1..4   -> LN(elu(x W1_b^T + b1_b) W2_b^T + b2_b) * gamma_b + beta_b

Strategy:
- Host routing (metadata only): tokens are sorted by branch and each
  branch's token list is split evenly over the 8 cores, so every core runs
  an identical-shape grouped FFN over ~1/8 of the routed tokens. Gather of
  x and the final scatter are host-side shard/unshard steps.
- All matmul operands in bf16 (fp32 PSUM accumulation): halves HBM+SBUF
  traffic and enables fast-weight-load so LDWEIGHTS mostly hides behind
  matmuls (the fp32r baseline exposed ~40us of weight loads). rel err of
  the result ~4e-3, well within the 2e-2 budget.
- ELU = max(v,0) + (min(exp(v),1) - 1): ScalarE exp (PSUM->SBUF bf16),
  DVE tensor_scalar min/sub in bf16 (4x mode), DVE scalar_tensor_tensor
  max/add reading v from PSUM, writing h in bf16. (GPSIMD stock
  elementwise measured ~14 cyc/elem - unusable for bulk work.)
- LayerNorm: mean arrives free as a 257th output column of the W2 matmul
  (host appends colsum(W2)/256); variance = Sum(y^2)/256 - mu^2 with the
  squared sum from one ScalarE Square+accum_out pass per token tile;
  rstd computed as exp(-0.5*ln(var+eps)) so every ScalarE function
  (exp/ln/square) lives in ONE activation table set - zero table swaps
  (the sqrt-based baseline paid ~13us in ACT_TABLE_LOADs). The normalize
  is a single fused (y - mu)*rstd tensor_scalar per token tile, with mu
  read directly out of PSUM as a per-partition scalar AP. The rstd chain
  runs per token-tile PAIR so PSUM banks recycle early.
- Chunk-level software pipeline: W2+LN of chunk c-1 is emitted interleaved
  with W1+ELU of chunk c so the Tensor engine always has independent work
  while the elementwise chain drains; x/weights DMAs prefetch one chunk
  ahead.
- Engine balance measured on HW: DVE ~122us (bottleneck), Tensor ~107us
  (~89us of matmul rows + LDW/ramp), ScalarE ~104us. Attempts to offload
  DVE work to ScalarE (finals, m2, relu-path ELU) all REGRESSED: the
  in-order ACT queue stalls on chain-dependent ops and starves the exp
  stream. Keep ScalarE's stream homogeneous (exp/Square only).
"""

import json
import os

import numpy as np

B, T = 32, 2048
D_MODEL = 256
D_FF = 1024
N_B = 4
NCORES = 8
LN_EPS = 1e-12
NTOK = B * T

# ---------------------------------------------------------------------------
# walrus workaround: this container's compiler accepts at most one sync wait
# per CTRL-class instruction; split extras onto NoOp carriers.
# ---------------------------------------------------------------------------


def _split_excess_waits(bir: dict, max_waits: int = 1) -> None:
    for fn in bir.get("functions", []):
        for blk in fn.get("blocks", []):
            insts = blk.get("instructions")
            if not insts:
                continue
            new = []
            for inst in insts:
                si = inst.get("sync_info")
                waits = (si or {}).get("on_wait") or []
                if len(waits) > max_waits:
                    excess, keep = waits[:-max_waits], waits[-max_waits:]
                    for k, w in enumerate(excess):
                        new.append(
                            {
                                "debug": inst.get("debug", 0),
                                "engine": inst["engine"],
                                "ins": [],
                                "name": f"{inst['name']}-wsplit{k}",
                                "opcode": "NoOp",
                                "outs": [],
                                "sync_info": {"on_update": [], "on_wait": [w]},
                            }
                        )
                    si["on_wait"] = keep
                new.append(inst)
            blk["instructions"] = new


_bir_fix_installed = False


def _install_bir_fix():
    global _bir_fix_installed
    if _bir_fix_installed:
        return
    import concourse.bass_utils as bass_utils
    import concourse.bass2jax as bass2jax

    orig = bass_utils.compile_bir_kernel

    if os.environ.get("LDW_OPT"):
        _orig_bvo = bass_utils.bir_verify_and_optimise

        def _bvo(tmpdir, inp="bir.json", outp="file.neff", arch=None, **kw):
            import unittest.mock as _mock

            real_run = bass_utils.run_command

            def run2(argv, **kwargs):
                argv = [
                    a.replace("--enable-ldw-opt=false", "--enable-ldw-opt=true")
                    for a in argv
                ]
                return real_run(argv, **kwargs)

            with _mock.patch.object(bass_utils, "run_command", run2):
                return _orig_bvo(tmpdir, inp, outp, arch, **kw)

        bass_utils.bir_verify_and_optimise = _bvo

    def patched(bir_json, tmpdir, neff_name="file.neff"):
        bir = json.loads(bir_json)
        _split_excess_waits(bir)
        return orig(json.dumps(bir).encode(), tmpdir, neff_name)

    bass_utils.compile_bir_kernel = patched
    bass2jax.compile_bir_kernel = patched

    # Synthesize antenv.axon_hooks (absent in this image) so trace=True can
    # reach the terminal's NTFF profiler via the axon .so.
    import sys
    import types

    if "antenv.axon_hooks" not in sys.modules:
        try:
            from trn_agent_boot.trn_boot import _ntff_profile_via_ctypes

            hook = _ntff_profile_via_ctypes("/opt/axon/libaxon_pjrt.so")
            mod = types.ModuleType("antenv.axon_hooks")
            mod.get_axon_ntff_profile_hook = lambda: hook
            mod.set_axon_ntff_profile_hook = lambda h: None
            sys.modules["antenv.axon_hooks"] = mod
        except Exception:
            pass
    _bir_fix_installed = True


# ---------------------------------------------------------------------------
# device kernel builder
# ---------------------------------------------------------------------------

_BUILD_CACHE = {}


def _chunks(cap, w=512):
    out = []
    off = 0
    while off < cap:
        out.append((off, min(w, cap - off)))
        off += w
    return out


def _build(caps, b1_nonzero, b2_nonzero, gb_nontrivial):
    kedt = os.environ.get("KEDT", "bf16")  # dtype of exp/neg intermediates
    kfs = int(os.environ.get("KFS", "0"))  # finals per chunk on ScalarE
    ka2 = int(os.environ.get("KA2", "0"))  # ELU groups per chunk on relu path
    key = (tuple(caps), b1_nonzero, b2_nonzero, gb_nontrivial, kedt, kfs, ka2)
    if key in _BUILD_CACHE:
        return _BUILD_CACHE[key]

    import concourse.bass as bass
    import concourse.tile as tile
    from concourse import mybir

    f32 = mybir.dt.float32
    bf16 = mybir.dt.bfloat16
    edt = bf16 if kedt == "bf16" else f32
    S = sum(caps)
    KD = D_MODEL // 128  # 2 chunks of the model dim
    KF = D_FF // 128  # 8 chunks of the ff dim
    D2 = D_MODEL + 2  # mean column + pad

    nc = bass.Bass("TRN2")
    xg = nc.dram_tensor("xg", [KD, 128, S], bf16, kind="ExternalInput")
    w1t = nc.dram_tensor("w1t", [N_B, KD, 128, D_FF], bf16, kind="ExternalInput")
    w2t = nc.dram_tensor("w2t", [N_B, KF, 128, D2], bf16, kind="ExternalInput")
    if b2_nonzero:
        b2e = nc.dram_tensor("b2e", [N_B, D2], bf16, kind="ExternalInput")
    if b1_nonzero:
        b1d = nc.dram_tensor("b1", [N_B, D_FF], bf16, kind="ExternalInput")
    if gb_nontrivial:
        gamd = nc.dram_tensor("gamma", [N_B, D_MODEL], f32, kind="ExternalInput")
        betd = nc.dram_tensor("beta", [N_B, D_MODEL], f32, kind="ExternalInput")
    yc = nc.dram_tensor("yc", [S, D_MODEL], bf16, kind="ExternalOutput")

    AF = mybir.ActivationFunctionType
    OP = mybir.AluOpType

    # flat chunk list across branches
    descs = []
    seg_off = 0
    for n in range(N_B):
        cap = caps[n]
        if cap == 0:
            continue
        for off, W in _chunks(cap):
            descs.append((n, off, W, seg_off + off, off == 0))
        seg_off += cap

    with tile.TileContext(nc) as tc:
        with (
            tc.tile_pool(name="singles", bufs=1) as singles,
            tc.tile_pool(name="w1p", bufs=2) as w1p,
            tc.tile_pool(name="w2p", bufs=2) as w2p,
            tc.tile_pool(name="cns", bufs=2) as cns,
            tc.tile_pool(name="xp", bufs=4) as xp,
            tc.tile_pool(name="ep", bufs=4) as ep,
            tc.tile_pool(name="ngp", bufs=4) as ngp,
            tc.tile_pool(name="hp", bufs=3) as hp,
            tc.tile_pool(name="op_", bufs=4) as op_,
            tc.tile_pool(name="scrp", bufs=2) as scrp,
            tc.tile_pool(name="stp", bufs=6) as stp,
            tc.tile_pool(name="php", bufs=2, space="PSUM") as php,
            tc.tile_pool(name="pyp", bufs=4, space="PSUM") as pyp,
        ):
            eps_tile = singles.tile([128, 1], f32)
            nc.vector.memset(eps_tile, LN_EPS)
            if b1_nonzero:
                ones_row = singles.tile([1, 512], bf16)
                nc.vector.memset(ones_row, 1.0)
            if b2_nonzero:
                ones_col = singles.tile([1, 128], bf16)
                nc.vector.memset(ones_col, 1.0)

            bstate = {}  # branch -> weight tiles
            xg_tiles = {}  # chunk idx -> xg tile

            def emit_weights(i):
                if i >= len(descs):
                    return
                n, off, W, goff, first = descs[i]
                if first:
                    w1_sb = w1p.tile([128, KD, D_FF], bf16, tag="w1")
                    for k in range(KD):
                        nc.sync.dma_start(out=w1_sb[:, k, :], in_=w1t[n, k])
                    b1_sb = b2e_sb = gam_bc = bet_bc = None
                    if b1_nonzero:
                        b1_sb = cns.tile([1, D_FF], bf16, tag="b1")
                        nc.sync.dma_start(out=b1_sb, in_=b1d[n : n + 1, :])
                    if b2_nonzero:
                        b2e_sb = cns.tile([1, D2], bf16, tag="b2e")
                        nc.sync.dma_start(out=b2e_sb, in_=b2e[n : n + 1, :])
                    if gb_nontrivial:
                        gam_bc = cns.tile([128, D_MODEL], f32, tag="gam")
                        bet_bc = cns.tile([128, D_MODEL], f32, tag="bet")
                        gsrc = gamd[n : n + 1, :]
                        bsrc = betd[n : n + 1, :]
                        nc.gpsimd.dma_start(
                            out=gam_bc,
                            in_=bass.AP(
                                tensor=gsrc.tensor,
                                offset=gsrc.offset,
                                ap=[[0, 128], gsrc.ap[1]],
                            ),
                        )
                        nc.gpsimd.dma_start(
                            out=bet_bc,
                            in_=bass.AP(
                                tensor=bsrc.tensor,
                                offset=bsrc.offset,
                                ap=[[0, 128], bsrc.ap[1]],
                            ),
                        )
                    bstate[n] = [w1_sb, None, b1_sb, b2e_sb, gam_bc, bet_bc]

            def emit_w2(i):
                if i >= len(descs):
                    return
                n, off, W, goff, first = descs[i]
                if first:
                    w2_sb = w2p.tile([128, KF, D2], bf16, tag="w2")
                    nc.sync.dma_start(
                        out=w2_sb, in_=w2t[n].rearrange("j p d -> p j d")
                    )
                    bstate[n][1] = w2_sb

            def emit_x(i):
                if i >= len(descs):
                    return
                n, off, W, goff, first = descs[i]
                xg_sb = xp.tile([128, KD, 512], bf16, tag="xg")
                nc.sync.dma_start(
                    out=xg_sb[:, :, :W],
                    in_=xg[:, :, goff : goff + W].rearrange("k p w -> p k w"),
                )
                xg_tiles[i] = xg_sb

            def phase1(i):
                """W1 matmuls + ELU for chunk i; yields once per f-group."""
                emit_weights(i + 1)
                emit_x(i + 1)
                emit_w2(i + 1)
                n, off, W, goff, first = descs[i]
                w1_sb, w2_sb, b1_sb, b2e_sb, gam_bc, bet_bc = bstate[n]
                xg_sb = xg_tiles.pop(i)
                h_sb = hp.tile([128, KF, 512], bf16, tag="h")
                for g in range(KF // 2):
                    ph = php.tile([128, 2, 512], f32, tag="ph")
                    for fi in range(2):
                        f = g * 2 + fi
                        fs = slice(f * 128, (f + 1) * 128)
                        nc.tensor.matmul(
                            ph[:, fi, :W],
                            w1_sb[:, 0, fs],
                            xg_sb[:, 0, :W],
                            start=True,
                            stop=False,
                        )
                        nc.tensor.matmul(
                            ph[:, fi, :W],
                            w1_sb[:, 1, fs],
                            xg_sb[:, 1, :W],
                            start=False,
                            stop=not b1_nonzero,
                        )
                        if b1_nonzero:
                            nc.tensor.matmul(
                                ph[:, fi, :W],
                                b1_sb[:, fs],
                                ones_row[:, :W],
                                start=False,
                                stop=True,
                            )
                    # elu(v) = max(v,0) + (min(exp(v),1) - 1)
                    e_sb = ep.tile([128, 2, 512], edt, tag="e")
                    nc.scalar.activation(e_sb[:, :, :W], ph[:, :, :W], AF.Exp)
                    ng_sb = ngp.tile([128, 2, 512], edt, tag="ng")
                    nc.vector.tensor_scalar(
                        ng_sb[:, :, :W],
                        e_sb[:, :, :W],
                        scalar1=1.0,
                        scalar2=1.0,
                        op0=OP.min,
                        op1=OP.subtract,
                    )
                    nc.vector.scalar_tensor_tensor(
                        h_sb[:, g * 2 : g * 2 + 2, :W],
                        ph[:, :, :W],
                        0.0,
                        ng_sb[:, :, :W],
                        op0=OP.max,
                        op1=OP.add,
                    )
                    yield
                xg_tiles[("h", i)] = h_sb

            def phase2(i):
                """W2 matmuls + LN + store for chunk i; yields per token tile."""
                n, off, W, goff, first = descs[i]
                w1_sb, w2_sb, b1_sb, b2e_sb, gam_bc, bet_bc = bstate[n]
                h_sb = xg_tiles.pop(("h", i))
                nW = W // 128
                mu_neg = stp.tile([128, 4], f32, tag="mu")
                varall = stp.tile([128, 4], f32, tag="var")
                aux = stp.tile([128, 12], f32, tag="aux")
                m2, tv, lnv = aux[:, 0:4], aux[:, 4:8], aux[:, 8:12]
                rstd = stp.tile([128, 4], f32, tag="rstd")
                murstd = stp.tile([128, 4], f32, tag="mrs")
                o2 = op_.tile([128, 4, D_MODEL], bf16, tag="o2")
                pys = []
                for t in range(nW):
                    py = pyp.tile([128, 512], f32, tag="py")
                    for f in range(KF):
                        nc.tensor.matmul(
                            py[:, :D2],
                            h_sb[:, f, t * 128 : (t + 1) * 128],
                            w2_sb[:, f, :],
                            start=(f == 0),
                            stop=not b2_nonzero and f == KF - 1,
                        )
                    if b2_nonzero:
                        nc.tensor.matmul(
                            py[:, :D2],
                            ones_col[:, :128],
                            b2e_sb,
                            start=False,
                            stop=True,
                        )
                    scr = scrp.tile([128, D_MODEL], bf16, tag="scr")
                    nc.scalar.activation(
                        scr,
                        py[:, :D_MODEL],
                        AF.Square,
                        accum_out=varall[:, t : t + 1],
                    )
                    # m2 = -256 * mu^2 (mu read straight out of PSUM col 256)
                    nc.vector.tensor_scalar(
                        m2[:, t : t + 1],
                        py[:, D_MODEL : D_MODEL + 1],
                        scalar1=py[:, D_MODEL : D_MODEL + 1],
                        scalar2=-float(D_MODEL),
                        op0=OP.mult,
                        op1=OP.mult,
                    )
                    if t < kfs:
                        nc.vector.tensor_scalar_mul(
                            mu_neg[:, t : t + 1],
                            py[:, D_MODEL : D_MODEL + 1],
                            scalar1=-1.0,
                        )
                    pys.append(py)
                    # finish LN for a pair of token tiles as soon as their
                    # stats are in, so their PSUM banks free up early.
                    if t % 2 == 1 or t == nW - 1:
                        lo = t & ~1
                        hi = t + 1
                        sl = slice(lo, hi)
                        # rstd = exp(-0.5 * ln(sum(y^2)/256 - mu^2 + eps))
                        nc.vector.tensor_tensor(
                            tv[:, sl], varall[:, sl], m2[:, sl], OP.add
                        )
                        nc.scalar.activation(
                            lnv[:, sl],
                            tv[:, sl],
                            AF.Ln,
                            bias=eps_tile,
                            scale=1.0 / D_MODEL,
                        )
                        nc.scalar.activation(
                            rstd[:, sl], lnv[:, sl], AF.Exp, scale=-0.5
                        )
                        if kfs > 0:
                            nc.gpsimd.tensor_tensor(
                                murstd[:, sl], mu_neg[:, sl], rstd[:, sl], OP.mult
                            )
                        for u in range(lo, hi):
                            if u < kfs:
                                nc.scalar.activation(
                                    o2[:, u, :],
                                    pys[u][:, :D_MODEL],
                                    AF.Identity,
                                    bias=murstd[:, u : u + 1],
                                    scale=rstd[:, u : u + 1],
                                )
                            else:
                                nc.vector.tensor_scalar(
                                    o2[:, u, :],
                                    pys[u][:, :D_MODEL],
                                    scalar1=pys[u][:, D_MODEL : D_MODEL + 1],
                                    scalar2=rstd[:, u : u + 1],
                                    op0=OP.subtract,
                                    op1=OP.mult,
                                )
                            if gb_nontrivial:
                                nc.vector.tensor_mul(
                                    o2[:, u, :], o2[:, u, :], gam_bc
                                )
                                nc.vector.tensor_add(
                                    o2[:, u, :], o2[:, u, :], bet_bc
                                )
                    yield
                nc.sync.dma_start(
                    out=yc[goff : goff + W, :].rearrange("(c p) d -> p c d", p=128),
                    in_=o2[:, :nW, :],
                )
                yield

            emit_weights(0)
            emit_x(0)
            emit_w2(0)
            prev = None
            for i in range(len(descs)):
                p1 = phase1(i)
                if prev is None:
                    for _ in p1:
                        pass
                else:
                    p2 = phase2(prev)
                    gens = [p2, p1]
                    gi = 0
                    while gens:
                        g = gens[gi % len(gens)]
                        try:
                            next(g)
                            gi += 1
                        except StopIteration:
                            gens.remove(g)
                prev = i
            for _ in phase2(prev):
                pass

    _BUILD_CACHE[key] = nc
    return nc


# ---------------------------------------------------------------------------
# host wrapper
# ---------------------------------------------------------------------------


def kernel(x, b_seq, w1, b1, w2, b2, gamma, beta):
    _install_bir_fix()
    import ml_dtypes
    from concourse.bass_utils import run_bass_kernel_spmd

    bf16 = ml_dtypes.bfloat16
    x = np.asarray(x, dtype=np.float32)
    b_seq = np.asarray(b_seq, dtype=np.int32)
    w1 = np.asarray(w1, dtype=np.float32)
    b1 = np.asarray(b1, dtype=np.float32)
    w2 = np.asarray(w2, dtype=np.float32)
    b2 = np.asarray(b2, dtype=np.float32)
    gamma = np.asarray(gamma, dtype=np.float32)
    beta = np.asarray(beta, dtype=np.float32)

    x_flat = x.reshape(NTOK, D_MODEL)
    bs = b_seq.reshape(NTOK)

    # token ids per branch, split evenly over cores
    parts = []  # parts[n][c] -> int array of token ids
    for n in range(1, N_B + 1):
        idx = np.nonzero(bs == n)[0].astype(np.int64)
        parts.append(np.array_split(idx, NCORES))
    caps = []
    for n in range(N_B):
        mx = max(len(p) for p in parts[n])
        caps.append(0 if mx == 0 else ((mx + 127) // 128) * 128)
    S = sum(caps)
    D2 = D_MODEL + 2

    b1_nonzero = bool(np.any(b1))
    b2_nonzero = bool(np.any(b2))
    gb_nontrivial = bool(np.any(beta)) or not bool(np.all(gamma == 1.0))

    nc = _build(tuple(caps), b1_nonzero, b2_nonzero, gb_nontrivial)

    # weight layouts
    w1t = np.ascontiguousarray(
        w1.transpose(0, 2, 1).reshape(N_B, D_MODEL // 128, 128, D_FF)
    ).astype(bf16)
    w2t_core = w2.transpose(0, 2, 1).reshape(N_B, D_FF // 128, 128, D_MODEL)
    w2_colmean = (w2.sum(axis=1) / D_MODEL).reshape(N_B, D_FF // 128, 128, 1)
    w2t = np.ascontiguousarray(
        np.concatenate(
            [w2t_core, w2_colmean, np.zeros_like(w2_colmean)], axis=3
        )
    ).astype(bf16)
    b2e = np.ascontiguousarray(
        np.concatenate(
            [
                b2,
                b2.sum(axis=1, keepdims=True) / D_MODEL,
                np.zeros((N_B, 1), np.float32),
            ],
            axis=1,
        )
    ).astype(bf16)

    in_maps = []
    for c in range(NCORES):
        gidx = np.zeros(S, dtype=np.int64)
        seg = 0
        for n in range(N_B):
            p = parts[n][c]
            gidx[seg : seg + len(p)] = p
            seg += caps[n]
        xgc = np.ascontiguousarray(
            x_flat[gidx].T.reshape(D_MODEL // 128, 128, S)
        ).astype(bf16)
        m = {"xg": xgc, "w1t": w1t, "w2t": w2t}
        if b2_nonzero:
            m["b2e"] = b2e
        if b1_nonzero:
            m["b1"] = np.ascontiguousarray(b1).astype(bf16)
        if gb_nontrivial:
            m["gamma"] = gamma
            m["beta"] = beta
        in_maps.append(m)

    import time

    trace = bool(os.environ.get("KERNEL_TRACE"))
    res = None
    for attempt in range(3):
        try:
            res = run_bass_kernel_spmd(
                nc, in_maps, core_ids=list(range(NCORES)), trace=trace
            )
            break
        except Exception:
            # transient NRT device errors have been observed on the first
            # execution of a freshly compiled NEFF; retry
            if attempt == 2:
                raise
            time.sleep(3)
    global LAST_RESULTS
    LAST_RESULTS = res

    out_flat = np.zeros((NTOK, D_MODEL), dtype=np.float32)
    for c in range(NCORES):
        ycc = np.asarray(res.results[c]["yc"]).astype(np.float32)
        seg = 0
        for n in range(N_B):
            p = parts[n][c]
            out_flat[p] = ycc[seg : seg + len(p)]
            seg += caps[n]
    return out_flat.reshape(B, T, D_MODEL)
